# revision 48
# baseline (speedup 1.0000x reference)
"""Causal multi-head attention block (B=2, T=2048, C=1024, H=16) on 8 TRN2 cores.

Sharding: tensor-parallel over heads x data-parallel over batch.
Core c handles batch b = c // 4 and head-group hg = c % 4 (4 heads = 256 of
the 1024 channel columns). Each core computes, for its batch and heads:
    QT/KT = (Wslice/8)^T X^T + b/8   (scores pre-scaled by 1/sqrt(D))
    V     = X Wv_slice + bv          (bias added on DVE during evacuation)
    S^T   = K Q^T (causal, streamed in 128x512 chunks), P = exp(S^T) in bf16
    O[qj] = sum_jc P(jc,qj)^T [V|1]  per 128-query chunk ("flipped" AV: V is
            the moving operand -> 65 PE rows per 128x128 block), normalized
            per partition by the softmax sum in column 64, then PE-transposed
            back to [dims, queries]
    partial = O^T normalized -> @ Wo_rows_slice   [2048, 1024] in bf16
Host sums the 4 bf16 partials per batch in fp64 and adds bo.

Everything on the PE datapath runs in bf16 (1 cycle/row at any output width,
half the DMA bytes); PSUM accumulation stays fp32. Bulk loads stream on the
otherwise-idle Pool DMA queue so exp dispatch on the Activation queue never
delays them; projections and the O-projection are woven between attention
tiles so exp-bound windows keep the PE fed.
"""

from contextlib import ExitStack

import ml_dtypes
import numpy as np

import concourse.bacc as bacc
import concourse.mybir as mybir
import concourse.tile as tile
from concourse.bass_utils import run_bass_kernel_spmd

B, T, C, H, D = 2, 2048, 1024, 16, 64
N_CORES = 8
HG = 4                  # head-groups (tensor parallel)
HPC = H // HG           # heads per core = 4
HD = HPC * D            # channel slice per core = 256
P = 128                 # partitions
NT = T // 512           # 4 i-tiles of 512
NIC = T // P            # 16 i-chunks of 128
NKC = C // P            # 8 contraction chunks of 128
F32 = mybir.dt.float32
F32R = mybir.dt.float32r
BF16 = mybir.dt.bfloat16
AF = mybir.ActivationFunctionType

MM_DT = BF16            # matmul datapath dtype
NP_DT = ml_dtypes.bfloat16

_CACHE: dict = {}


def _build_program():
    nc = bacc.Bacc("TRN2", debug=False)

    XT = nc.dram_tensor("XT", [C, T], MM_DT, kind="ExternalInput").ap()
    WQKV = nc.dram_tensor("WQKV", [C, 3 * HD], MM_DT, kind="ExternalInput").ap()
    BQK = nc.dram_tensor("BQK", [P, 4], F32, kind="ExternalInput").ap()
    BV = nc.dram_tensor("BV", [1, HD], MM_DT, kind="ExternalInput").ap()
    WO = nc.dram_tensor("WO", [HD, C], MM_DT, kind="ExternalInput").ap()
    OUT = nc.dram_tensor("OUT", [T, C], MM_DT, kind="ExternalOutput").ap()

    # Causal chunk mask: for diagonal chunk k (k=0..3), valid iff f >= p + 128k,
    # realized as slices of Mbig[p, x] = (x >= p + 384).
    mb = (np.arange(512)[None, :] >= np.arange(P)[:, None]).astype(NP_DT)
    MBIG = nc.inline_tensor(mb, name="mbig").ap()
    IDN = nc.inline_tensor(np.eye(P, dtype=NP_DT), name="idn").ap()

    with tile.TileContext(nc) as tc:
        _trace_kernel(tc, XT, WQKV, BQK, BV, WO, OUT, MBIG, IDN)
    nc.compile()
    return nc


def _trace_kernel(tc, XT, WQKV, BQK, BV, WO, OUT, MBIG, IDN):
    nc = tc.nc

    with ExitStack() as ctx:
        consts = ctx.enter_context(tc.tile_pool(name="consts", bufs=1))
        wpool = ctx.enter_context(tc.tile_pool(name="weights", bufs=1))
        xpool = ctx.enter_context(tc.tile_pool(name="xt", bufs=1))
        qkv = ctx.enter_context(tc.tile_pool(name="qkv", bufs=1))

        # DMA queues: SP (nc.sync) and Pool (nc.gpsimd). Pool's SEQ is
        # otherwise idle, so bulk loads never queue behind exp dispatch the
        # way they would on the Activation queue.
        qs, qa = nc.sync, nc.gpsimd

        # ---- tiles ----
        mbig_sb = consts.tile([P, 512], MM_DT, name="mbig_sb")
        # row 0 of mbig is all-ones; reuse it where a ones vector is needed
        ones_sb = mbig_sb[0:1, 0:P]
        bias_sb = consts.tile([P, 4], F32, name="bias_sb")  # bq m0,m1, bk m0,m1
        bvb_sb = consts.tile([P, HD], MM_DT, name="bvb_sb")
        idn_sb = consts.tile([P, P], MM_DT, name="idn_sb")
        wqkv_sb = wpool.tile([P, NKC, 3 * HD], MM_DT, name="wqkv_sb")
        wo_sb = wpool.tile([P, 2, C], MM_DT, name="wo_sb")
        xts = [
            xpool.tile([P, T], MM_DT, name=f"xt{kc}", tag=f"xt{kc}")
            for kc in range(NKC)
        ]
        qt_sb = [qkv.tile([P, T], MM_DT, name=f"qt{m}", tag=f"qt{m}") for m in range(2)]
        kt_sb = [qkv.tile([P, T], MM_DT, name=f"kt{m}", tag=f"kt{m}") for m in range(2)]
        v_sb = qkv.tile([P, NIC, HPC, D + 1], MM_DT, name="v_sb")
        ot_sb = [qkv.tile([P, T], MM_DT, name=f"ot{m}", tag=f"ot{m}") for m in range(2)]

        def wq_c(kc, msl):
            return wqkv_sb[:, kc, msl]

        def wk_c(kc, msl):
            return wqkv_sb[:, kc, slice(HD + msl.start, HD + msl.stop)]

        def wv_c(kc):
            return wqkv_sb[:, kc, 2 * HD : 3 * HD]

        # Preload the ACT Exp table while the first DMAs stream (the table
        # load costs ~1.3us and would otherwise land on the first real exp).
        scx = consts.tile([1, 1], F32, name="scx")
        nc.vector.memset(scx, 0.0)
        scy = consts.tile([1, 1], F32, name="scy")
        nc.scalar.activation(scy, scx, AF.Exp)

        # ---- loads ----
        # Weights and the t=0 column-block of X^T stream first (kc-ordered,
        # alternating across both HWDGE queues) so t=0 projections complete
        # after ~5MB; remaining X^T column-blocks stream per i-tile behind
        # them and the whole pipeline becomes PE-paced after ~15us.
        nc.gpsimd.memset(v_sb[:, :, :, D : D + 1], 1.0)
        # kc=0 pieces go via SP/HWDGE (fast path) so the first projection
        # matmuls start ~2.4us in; bulk streams via the idle Pool queue.
        qs.dma_start(wqkv_sb[:, 0, 0:HD], WQKV[0:P, 0:HD])
        qs.dma_start(xts[0][:, 0:256], XT[0:P, 0:256])
        qs.dma_start(xts[0][:, 256:512], XT[0:P, 256:512])
        qs.dma_start(wqkv_sb[:, 0, HD : 3 * HD], WQKV[0:P, HD : 3 * HD])
        qs.dma_start(xts[1][:, 0:512], XT[P : 2 * P, 0:512])
        qa.dma_start(wqkv_sb[:, 1, :], WQKV[P : 2 * P, :])
        for kcp in range(1, 4):  # remaining weight chunks in kc-pairs
            kc = 2 * kcp
            qa.dma_start(
                wqkv_sb[:, kc : kc + 2, :],
                WQKV[kc * P : (kc + 2) * P, :].rearrange("(a p) c -> p a c", p=P),
            )
            qs.dma_start(xts[kc][:, 0:512], XT[kc * P : (kc + 1) * P, 0:512])
            qx = qs if kc % 2 == 0 else qa
            qx.dma_start(
                xts[kc + 1][:, 0:512], XT[(kc + 1) * P : (kc + 2) * P, 0:512]
            )
            if kc == 2:
                qs.dma_start(mbig_sb, MBIG)
                qs.dma_start(bias_sb, BQK)
            if kc == 4:
                qs.dma_start(bvb_sb, BV.to_broadcast((P, HD)))
        for t in range(1, NT):
            sl_ = slice(512 * t, 512 * (t + 1))
            for kc in range(NKC):
                qx = qs if (kc + t) % 2 == 0 else qa
                qx.dma_start(xts[kc][:, sl_], XT[kc * P : (kc + 1) * P, sl_])
            if t == 1:
                qa.dma_start(wo_sb, WO.rearrange("(a p) c -> p a c", p=P))
                qs.dma_start(idn_sb, IDN)

        # Single PSUM pool; stages share tag families so the Tile scheduler can
        # pipeline projections, attention and output projection freely.
        # Banks: big 2x2 + pot 2x1 + sm 2x1 = 8.
        psum = ctx.enter_context(tc.tile_pool(name="psum", bufs=2, space="PSUM"))
        spool = ctx.enter_context(tc.tile_pool(name="spool", bufs=9))
        npool = ctx.enter_context(tc.tile_pool(name="npool", bufs=3))
        opool = ctx.enter_context(tc.tile_pool(name="opool", bufs=4))

        # ---- stage A: projections for one i-tile ----
        def stage_a(t):
            sl = slice(512 * t, 512 * (t + 1))
            for m in range(2):
                msl = slice(P * m, P * (m + 1))
                pqk = psum.tile([P, 1024], F32, tag="big", bufs=3)
                for kc in range(NKC):
                    if t == 0 and kc == 0:
                        for h0, h1 in ((0, 256), (256, 512)):
                            nc.tensor.matmul(
                                pqk[:, h0:h1],
                                lhsT=wq_c(kc, msl),
                                rhs=xts[kc][:, h0:h1],
                                start=True,
                                stop=False,
                                skip_group_check=True,
                            )
                            nc.tensor.matmul(
                                pqk[:, 512 + h0 : 512 + h1],
                                lhsT=wk_c(kc, msl),
                                rhs=xts[kc][:, h0:h1],
                                start=True,
                                stop=False,
                                skip_group_check=True,
                            )
                        continue
                    nc.tensor.matmul(
                        pqk[:, 0:512],
                        lhsT=wq_c(kc, msl),
                        rhs=xts[kc][:, sl],
                        start=(kc == 0),
                        stop=(kc == NKC - 1),
                        skip_group_check=(t == 0),
                    )
                    nc.tensor.matmul(
                        pqk[:, 512:1024],
                        lhsT=wk_c(kc, msl),
                        rhs=xts[kc][:, sl],
                        start=(kc == 0),
                        stop=(kc == NKC - 1),
                        skip_group_check=(t == 0),
                    )
                nc.vector.tensor_scalar_add(
                    qt_sb[m][:, sl], pqk[:, 0:512], bias_sb[:, m : m + 1]
                )
                nc.vector.tensor_scalar_add(
                    kt_sb[m][:, sl], pqk[:, 512:1024], bias_sb[:, 2 + m : 3 + m]
                )
            for ic in range(4 * t, 4 * (t + 1)):
                isl = slice(P * ic, P * (ic + 1))
                pv = psum.tile([P, HD], F32, tag="pot", bufs=2)
                for kc in range(NKC):
                    nc.tensor.matmul(
                        pv,
                        lhsT=xts[kc][:, isl],
                        rhs=wv_c(kc),
                        start=(kc == 0),
                        stop=(kc == NKC - 1),
                    )
                nc.vector.tensor_add(
                    v_sb[:, ic, :, 0:D],
                    pv.rearrange("p (h d) -> p h d", d=D),
                    bvb_sb.rearrange("p (h d) -> p h d", d=D),
                )

        # ---- stage B: attention for one i-tile ----
        # Scores S^T = K Q^T stream per 128-key chunk; AV is "flipped": for
        # each 128-query chunk j, O[j] = sum_jc ex(jc,qj)^T @ [V|1](jc)
        # accumulates in po4 [128q, 65] with V as the MOVING operand (65 PE
        # rows per 128x128 block - half the PE cost of streaming queries).
        # PSUM accumulation groups have full-bank (2KB) zero-region
        # granularity, so q-chunks run sequentially, ping-ponging 2 banks.
        # The softmax sum lands per PARTITION (col 64), so normalization is
        # a batched reciprocal + per-partition tensor_scalar after copying
        # to SBUF; a PE transpose restores the [dims, queries] layout that
        # the output projection needs as its stationary operand.
        def stage_b(t, heads=range(HPC)):
            sl = slice(512 * t, 512 * (t + 1))
            widths = {0: 512, 1: 384, 2: 256, 3: 128}
            for l in heads:
                mc, ro = l // 2, 64 * (l % 2)
                qrow = slice(ro, ro + 64)

                # full (unmasked) chunks, exp'd in pairs
                fulls = []
                pool_ = spool
                for jcp in range(2 * t):
                    ps = psum.tile([P, 1024], F32, tag="big", bufs=3)
                    for half in (0, 1):
                        jc = 2 * jcp + half
                        nc.tensor.matmul(
                            ps[:, 512 * half : 512 * (half + 1)],
                            lhsT=kt_sb[mc][qrow, P * jc : P * (jc + 1)],
                            rhs=qt_sb[mc][qrow, sl],
                            start=True,
                            stop=True,
                        )
                    ex = pool_.tile([P, 1024], MM_DT, tag="ex")
                    nc.scalar.activation(ex, ps, AF.Exp)
                    fulls.append(ex)

                # diagonal chunks k=0..3 (jc = 4t+k), truncated to the valid
                # i-range: width w = 512-128k covers queries [512t+128k, ...);
                # element x maps to query offset 128k+x, valid iff x >= p.
                # Chunk 4t+k is full for q-chunks j > k, diagonal for j == k.
                diags = []
                for ka, kb in ((0, 1), (2, 3)):
                    ps = psum.tile([P, 1024], F32, tag="big", bufs=3)
                    for half, k in ((0, ka), (1, kb)):
                        w = widths[k]
                        nc.tensor.matmul(
                            ps[:, 512 * half : 512 * half + w],
                            lhsT=kt_sb[mc][qrow, P * (4 * t + k) : P * (4 * t + k + 1)],
                            rhs=qt_sb[mc][qrow, 512 * (t + 1) - w : 512 * (t + 1)],
                            start=True,
                            stop=True,
                        )
                    wb = widths[kb]
                    ex = pool_.tile([P, 1024], MM_DT, tag="ex")
                    nc.scalar.activation(ex[:, 0 : 512 + wb], ps[:, 0 : 512 + wb], AF.Exp)
                    for half, k in ((0, ka), (1, kb)):
                        # only x<128 of the region can violate causality
                        # (x >= p with p<128 holds for all x >= 128)
                        exh = ex[:, 512 * half : 512 * half + P]
                        nc.vector.tensor_mul(exh, exh, mbig_sb[:, 0:P])
                    diags.append(ex)

                # flipped AV, one q-chunk at a time (sequential PSUM groups)
                o4r = npool.tile([P, 4, D + 1], MM_DT, tag="o4r")
                for j in range(4):
                    po4 = psum.tile([P, D + 1], F32, tag="pot", bufs=2)
                    first = True
                    for jcp in range(2 * t):
                        for half in (0, 1):
                            nc.tensor.matmul(
                                po4,
                                lhsT=fulls[jcp][
                                    :, 512 * half + 128 * j : 512 * half + 128 * (j + 1)
                                ],
                                rhs=v_sb[:, 2 * jcp + half, l, 0 : D + 1],
                                start=first,
                                stop=False,
                                skip_group_check=True,
                            )
                            first = False
                    for k in range(j + 1):
                        w = widths[k]
                        col = 512 * (k % 2) + 128 * (j - k)
                        nc.tensor.matmul(
                            po4,
                            lhsT=diags[k // 2][:, col : col + 128],
                            rhs=v_sb[:, 4 * t + k, l, 0 : D + 1],
                            start=first,
                            stop=(k == j),
                            skip_group_check=True,
                        )
                        first = False
                    nc.vector.tensor_copy(o4r[:, j, :], po4)

                # batched normalization + PE transpose back to [dims, queries]
                rj = npool.tile([P, 4], F32, tag="rj")
                nc.vector.reciprocal(rj, o4r[:, :, D : D + 1])
                for j in range(4):
                    nc.vector.tensor_scalar_mul(
                        o4r[:, j, 0:D], o4r[:, j, 0:D], rj[:, j : j + 1]
                    )
                tr = psum.tile([D, 512], MM_DT, tag="pot", bufs=2)
                for j in range(4):
                    nc.tensor.transpose(
                        tr[:, 128 * j : 128 * (j + 1)], o4r[:, j, 0:D], idn_sb
                    )
                nc.vector.tensor_copy(ot_sb[mc][qrow, sl], tr)

        # ---- stage C: output projection for the 4 i-chunks of one i-tile ----
        def stage_c(t, ics=range(4), tail=False):
            for ic0 in ics:
                ic = 4 * t + ic0
                isl = slice(P * ic, P * (ic + 1))
                ob = opool.tile([P, C], MM_DT, tag="ob")
                for n in (0, 1):
                    po = psum.tile([P, 512], F32, tag="pot", bufs=2)
                    for kc in range(2):
                        nc.tensor.matmul(
                            po,
                            lhsT=ot_sb[kc][:, isl],
                            rhs=wo_sb[:, kc, 512 * n : 512 * (n + 1)],
                            start=(kc == 0),
                            stop=(kc == 1),
                        )
                    if tail and n == 0:
                        # exp stream is drained by now: parallel evacuation
                        # across ACT+DVE and both DMA-generation paths
                        # (SP/HWDGE + Pool/SWDGE) shortens the final drain.
                        nc.scalar.copy(ob[:, 0:512], po)
                    else:
                        nc.vector.tensor_copy(ob[:, 512 * n : 512 * (n + 1)], po)
                    qs.dma_start(
                        OUT[isl, 512 * n : 512 * (n + 1)],
                        ob[:, 512 * n : 512 * (n + 1)],
                    )

        # Emission order: weave ACT-free PE work (projections, O-projection)
        # into the exp-bound attention windows. b(t) needs a(t')<=t; c(t)
        # needs all of b(t). The last window (b1+c3+c1) is exp-light.
        stage_a(0)
        stage_b(0)
        stage_a(1)
        stage_a(2)
        stage_b(2, heads=(0, 1))
        stage_c(0, ics=(0, 1))
        stage_b(2, heads=(2, 3))
        stage_c(0, ics=(2, 3))
        stage_a(3)
        stage_b(3, heads=(0,))
        stage_c(2, ics=(0, 1))
        stage_b(3, heads=(1,))
        stage_c(2, ics=(2, 3))
        stage_b(3, heads=(2, 3))
        stage_c(3)
        stage_b(1)
        stage_c(1, tail=True)


def _get_program():
    if "nc" not in _CACHE:
        _CACHE["nc"] = _build_program()
    return _CACHE["nc"]


class _Runner:
    """Reusable SPMD executor (adapted from concourse.bass2jax.run_bass_via_pjrt)
    so repeated kernel() calls reuse one compiled executable."""

    def __init__(self, nc):
        import jax
        import concourse.mybir as mb
        from jax.sharding import Mesh, PartitionSpec
        from jax.experimental.shard_map import shard_map
        from concourse import bass2jax

        bass2jax.install_neuronx_cc_hook()
        self.jax = jax
        self.nc = nc
        partition_name = (
            nc.partition_id_tensor.name if nc.partition_id_tensor else None
        )
        in_names, out_names, out_avals, zero_outs = [], [], [], []
        for alloc in nc.m.functions[0].allocations:
            if not isinstance(alloc, mb.MemoryLocationSet):
                continue
            name = alloc.memorylocations[0].name
            if alloc.kind == "ExternalInput":
                if name != partition_name:
                    in_names.append(name)
            elif alloc.kind == "ExternalOutput":
                shape = tuple(alloc.tensor_shape)
                dtype = mb.dt.np(alloc.dtype)
                out_names.append(name)
                out_avals.append(jax.core.ShapedArray(shape, dtype))
                zero_outs.append((shape, dtype))
        self.n_params = len(in_names)
        self.in_names = list(in_names)
        self.out_names = out_names
        self.out_avals = out_avals
        self.zero_outs = zero_outs
        all_in_names = in_names + out_names + (
            [partition_name] if partition_name else []
        )
        donate = tuple(range(self.n_params, self.n_params + len(out_names)))

        def _body(*args):
            operands = list(args)
            if partition_name is not None:
                operands.append(bass2jax.partition_id_tensor())
            outs = bass2jax._bass_exec_p.bind(
                *operands,
                out_avals=tuple(out_avals),
                in_names=tuple(all_in_names),
                out_names=tuple(out_names),
                lowering_input_output_aliases=(),
                sim_require_finite=True,
                sim_require_nnan=True,
                nc=nc,
            )
            return tuple(outs)

        devices = jax.devices()[:N_CORES]
        self.mesh = Mesh(np.asarray(devices), ("core",))
        in_specs = (PartitionSpec("core"),) * (self.n_params + len(out_names))
        out_specs = (PartitionSpec("core"),) * len(out_names)
        self.sharded = jax.jit(
            shard_map(
                _body,
                mesh=self.mesh,
                in_specs=in_specs,
                out_specs=out_specs,
                check_rep=False,
            ),
            donate_argnums=donate,
            keep_unused=True,
        )

    def concat_inputs(self, in_maps):
        return [
            np.concatenate([np.asarray(m[name]) for m in in_maps], axis=0)
            for name in self.in_names
        ]

    def zeros(self):
        return [
            np.zeros((N_CORES * s[0], *s[1:]), d) for s, d in self.zero_outs
        ]

    def run(self, concat_in, zeros):
        out_arrs = self.sharded(*concat_in, *zeros)
        return out_arrs

    def split(self, out_arrs):
        res = []
        for c in range(N_CORES):
            res.append(
                {
                    name: np.asarray(out_arrs[i]).reshape(
                        N_CORES, *self.out_avals[i].shape
                    )[c]
                    for i, name in enumerate(self.out_names)
                }
            )
        return res


def _get_runner():
    if "runner" not in _CACHE:
        _CACHE["runner"] = _Runner(_get_program())
    return _CACHE["runner"]


def _shard_inputs(X, Wq, bq, Wk, bk, Wv, bv, Wo, bo):
    in_maps = []
    for c in range(N_CORES):
        b, hg = divmod(c, HG)
        cols = slice(HD * hg, HD * (hg + 1))
        bqk = np.stack(
            [
                bq[cols][:P] * 0.125,
                bq[cols][P:] * 0.125,
                bk[cols][:P],
                bk[cols][P:],
            ],
            axis=1,
        ).astype(np.float32)
        in_maps.append(
            {
                "XT": np.ascontiguousarray(X[b].T).astype(NP_DT),
                "WQKV": np.concatenate(
                    [Wq[:, cols] * 0.125, Wk[:, cols], Wv[:, cols]], axis=1
                ).astype(NP_DT),
                "BQK": bqk,
                "BV": bv[cols].reshape(1, HD).astype(NP_DT),
                "WO": np.ascontiguousarray(Wo[cols, :]).astype(NP_DT),
            }
        )
    return in_maps


def kernel(X, Wq, bq, Wk, bk, Wv, bv, Wo, bo):
    X = np.asarray(X, dtype=np.float32)
    Wq, bq = np.asarray(Wq, np.float32), np.asarray(bq, np.float32)
    Wk, bk = np.asarray(Wk, np.float32), np.asarray(bk, np.float32)
    Wv, bv = np.asarray(Wv, np.float32), np.asarray(bv, np.float32)
    Wo, bo = np.asarray(Wo, np.float32), np.asarray(bo, np.float32)

    runner = _get_runner()
    in_maps = _shard_inputs(X, Wq, bq, Wk, bk, Wv, bv, Wo, bo)
    res = runner.split(runner.run(runner.concat_inputs(in_maps), runner.zeros()))

    out = np.empty((B, T, C), dtype=np.float32)
    for b in range(B):
        acc = np.zeros((T, C), dtype=np.float64)
        for hg in range(HG):
            acc += res[HG * b + hg]["OUT"].astype(np.float64)
        out[b] = (acc + bo.astype(np.float64)).astype(np.float32)
    return out

